# revision 3
# baseline (speedup 1.0000x reference)
"""Trainium2 Bass kernel for nn_AttentionLayer (B=4, S=2048, D=1024, H=16).

Self-contained: builds and compiles an SPMD Bass/Tile program once, then
runs it across 8 NeuronCores via run_bass_kernel_spmd.

Sharding (no collectives): core c handles batch b = c // 2 and query-token
half c % 2 (1024 query tokens). Each core receives pre-transposed bf16
activations (x^T slices) plus bf16 weights, computes its [1024, 1024]
slice of the final layernorm output in fp32, and the host reassembles.

Per-core pipeline (all matmuls bf16 with fp32 PSUM accumulation):
- K / V projections as dense up-front TensorE phases (V in natural token-
  major layout with a per-head ones column so each head's attn@V matmul
  also produces its softmax denominator row).
- Attention processes heads sequentially: scores^T = Kh @ Qh^T into
  double-buffered PSUM, exp on ScalarE (scale=1/8 folded into the
  activation), attn@V accumulation; Q^T/residual projections are emitted
  one matmul per kb-step to fill TensorE slack inside the ACT-bound loop.
- Softmax normalization is deferred: denominators go to DRAM; per pair a
  broadcast-DMA + fast approximate reciprocal + one multiply normalizes
  the bf16 context off the critical path.
- FC + residual + layernorm finish per 128-token block.
"""

import numpy as np
import ml_dtypes


from contextlib import ExitStack

import concourse.bass as bass
import concourse.tile as tile
import concourse.mybir as mybir
from concourse import bacc

F32 = mybir.dt.float32
BF16 = mybir.dt.bfloat16
AF = mybir.ActivationFunctionType
ALU = mybir.AluOpType


def bcast_ap(ap: bass.AP, parts: int) -> bass.AP:
    """Partition-broadcast a [1, N]-shaped DRAM AP to [parts, N]."""
    return bass.AP(tensor=ap.tensor, offset=ap.offset,
                   ap=[[0, parts]] + list(ap.ap[-1:]))


def nsplits(total, cap=512):
    return [(i, min(cap, total - i)) for i in range(0, total, cap)]


def build(T=1024, S=2048, D=1024, H=16, DK=64, n_cores=8, eps=1e-5,
          trn_type="TRN2"):
    assert DK == 64 and H % 2 == 0 and D == H * DK
    DB = D // 128     # contraction chunks over d
    EB = D // 128     # e blocks (projection output chunks); == H//2
    TB = T // 128
    SB = S // 128
    PAIRS = H // 2
    VW = 65           # per-head vp stripe: 64 v columns + 1 ones column
    DEN_F = 2 * T // 128  # free size of the per-pair denominator tile

    nc = bacc.Bacc(trn_type, target_bir_lowering=False, debug=False,
                   num_devices=n_cores)

    qT = nc.dram_tensor("qT", [D, T], BF16, kind="ExternalInput").ap()
    kT = nc.dram_tensor("kT", [D, S], BF16, kind="ExternalInput").ap()
    vT = nc.dram_tensor("vT", [D, S], BF16, kind="ExternalInput").ap()
    Wq = nc.dram_tensor("Wq", [D, D], BF16, kind="ExternalInput").ap()
    Wk = nc.dram_tensor("Wk", [D, D], BF16, kind="ExternalInput").ap()
    Wv = nc.dram_tensor("Wv", [D, D], BF16, kind="ExternalInput").ap()
    Wfc = nc.dram_tensor("Wfc", [D, D], BF16, kind="ExternalInput").ap()
    bq = nc.dram_tensor("bq", [D], F32, kind="ExternalInput").ap()
    bk = nc.dram_tensor("bk", [D], F32, kind="ExternalInput").ap()
    bv = nc.dram_tensor("bv", [D], F32, kind="ExternalInput").ap()
    bfc = nc.dram_tensor("bfc", [D], F32, kind="ExternalInput").ap()
    gamma = nc.dram_tensor("gamma", [D], F32, kind="ExternalInput").ap()
    beta = nc.dram_tensor("beta", [D], F32, kind="ExternalInput").ap()
    out = nc.dram_tensor("out", [T, D], F32, kind="ExternalOutput").ap()

    den_dram = nc.dram_tensor("den_scratch", [H, T], F32).ap()
    qp_dram = nc.dram_tensor("qp_scratch", [T, D], F32).ap()
    rec_dram = nc.dram_tensor("rec_scratch", [H, T], F32).ap()

    with tile.TileContext(nc) as tc, ExitStack() as ctx:
        pconst = ctx.enter_context(tc.tile_pool(name="const", bufs=1))
        ppers = ctx.enter_context(tc.tile_pool(name="persist", bufs=1))

        # ---- tiny constants -------------------------------------------
        bqT = pconst.tile([128, EB], F32, tag="bqT", name="bqT")
        nc.sync.dma_start(out=bqT, in_=bq.rearrange("(e p) -> p e", p=128))
        bkT = pconst.tile([128, EB], F32, tag="bkT", name="bkT")
        nc.sync.dma_start(out=bkT, in_=bk.rearrange("(e p) -> p e", p=128))
        eps_t = pconst.tile([128, 1], F32, tag="eps", name="eps")
        nc.vector.memset(eps_t, eps)

        # ---- persistent outputs ---------------------------------------
        kpT_sb = [ppers.tile([128, S], BF16, tag=f"kpT{e}", name=f"kpT{e}")
                  for e in range(EB)]
        vp_sb = [ppers.tile([128, H * VW], BF16, tag=f"vp{s}", name=f"vp{s}")
                 for s in range(SB)]
        ctxT_sb = [ppers.tile([128, T], BF16, tag=f"ctxT{e}", name=f"ctxT{e}")
                   for e in range(EB)]

        pqx = ctx.enter_context(tc.tile_pool(name="qx", bufs=1))
        pwq = ctx.enter_context(tc.tile_pool(name="wq", bufs=1))
        qx_sb = [pqx.tile([128, T], BF16, tag=f"qx{d}", name=f"qx{d}")
                 for d in range(DB)]
        wq_sb = [pwq.tile([128, D], BF16, tag=f"wq{d}", name=f"wq{d}")
                 for d in range(DB)]

        # ================= K projection =================================
        # c-outer loop + per-half kx loads so the first matmul only waits
        # for wk + the first half of kT.
        with tc.tile_pool(name="wk", bufs=1) as pw, \
             tc.tile_pool(name="kx", bufs=1) as pkx, \
             tc.tile_pool(name="kps", bufs=3, space="PSUM") as pps:
            CK = min(S, 1024)
            NC_ = len(nsplits(S, CK))
            wk_sb = [pw.tile([128, D], BF16, tag=f"wk{d}", name=f"wk{d}")
                     for d in range(DB)]
            kx_sb = [[pkx.tile([128, CK], BF16, tag=f"kx{d}_{c}",
                               name=f"kx{d}_{c}") for c in range(NC_)]
                     for d in range(DB)]
            for d in range(DB):
                nc.sync.dma_start(out=wk_sb[d], in_=Wk[d * 128:(d + 1) * 128, :])
            for ci, (c0, cn) in enumerate(nsplits(S, CK)):
                for d in range(DB):
                    nc.sync.dma_start(out=kx_sb[d][ci][:, 0:cn],
                                      in_=kT[d * 128:(d + 1) * 128, c0:c0 + cn])
            # qx/wq load after kx (needed later, at attention start)
            for d in range(DB):
                nc.sync.dma_start(out=qx_sb[d], in_=qT[d * 128:(d + 1) * 128, :])
                nc.sync.dma_start(out=wq_sb[d], in_=Wq[d * 128:(d + 1) * 128, :])
            for ci, (c0, cn) in enumerate(nsplits(S, CK)):
                for e in range(EB):
                    ps = pps.tile([128, CK], F32, tag="kpT_ps", name="kpT_ps")
                    for d in range(DB):
                        for n0, nn in nsplits(cn):
                            nc.tensor.matmul(
                                ps[:, n0:n0 + nn],
                                lhsT=wk_sb[d][:, e * 128:(e + 1) * 128],
                                rhs=kx_sb[d][ci][:, n0:n0 + nn],
                                start=(d == 0), stop=(d == DB - 1))
                    nc.vector.tensor_scalar(
                        out=kpT_sb[e][:, c0:c0 + cn], in0=ps[:, 0:cn],
                        scalar1=bkT[:, e:e + 1], scalar2=None, op0=ALU.add)

        # ================= V projection (natural layout) ================
        with tc.tile_pool(name="wv", bufs=1) as pw, \
             tc.tile_pool(name="vx", bufs=1) as pvx, \
             tc.tile_pool(name="vbc", bufs=1) as pvbc, \
             tc.tile_pool(name="vps", bufs=3, space="PSUM") as pps:
            bv_bc = pvbc.tile([128, D], F32, tag="bv_bc", name="bv_bc")
            nc.gpsimd.dma_start(out=bv_bc, in_=bcast_ap(bv, 128))
            wv_sb = [pw.tile([128, D], BF16, tag=f"wv{d}", name=f"wv{d}")
                     for d in range(DB)]
            vx_sb = [pvx.tile([128, S], BF16, tag=f"vx{d}", name=f"vx{d}")
                     for d in range(DB)]
            for d in range(DB):
                nc.sync.dma_start(out=wv_sb[d], in_=Wv[d * 128:(d + 1) * 128, :])
                nc.sync.dma_start(out=vx_sb[d], in_=vT[d * 128:(d + 1) * 128, :])
            for s in range(SB):
                ps = pps.tile([128, D], F32, tag="vp_ps", name="vp_ps")
                for d in range(DB):
                    for n0, nn in nsplits(D):
                        nc.tensor.matmul(
                            ps[:, n0:n0 + nn],
                            lhsT=vx_sb[d][:, s * 128:(s + 1) * 128],
                            rhs=wv_sb[d][:, n0:n0 + nn],
                            start=(d == 0), stop=(d == DB - 1))
                vr = vp_sb[s].rearrange("p (h c) -> p h c", c=VW)
                nc.vector.tensor_add(
                    out=vr[:, :, 0:64],
                    in0=ps.rearrange("p (h c) -> p h c", c=DK),
                    in1=bv_bc.rearrange("p (h c) -> p h c", c=DK))
                nc.vector.memset(vr[:, :, 64:65], 1.0)

        # ================= attention (+ Q-proj, qp-nat interleaved) =====
        pwfc = ctx.enter_context(tc.tile_pool(name="wfc", bufs=1))
        wfc_sb = [pwfc.tile([128, D], BF16, tag=f"wfc{d}", name=f"wfc{d}")
                  for d in range(DB)]
        for d in range(DB):
            nc.gpsimd.dma_start(out=wfc_sb[d], in_=Wfc[d * 128:(d + 1) * 128, :])

        pbqfc = ctx.enter_context(tc.tile_pool(name="bqfcp", bufs=1))
        bqfc_bc = pbqfc.tile([128, D], F32, tag="bqfc", name="bqfc")
        nc.gpsimd.dma_start(out=bqfc_bc, in_=bcast_ap(bq, 128))
        tmp_bfc = pbqfc.tile([128, D], F32, tag="tmp_bfc", name="tmp_bfc")
        nc.gpsimd.dma_start(out=tmp_bfc, in_=bcast_ap(bfc, 128))
        nc.vector.tensor_add(out=bqfc_bc, in0=bqfc_bc, in1=tmp_bfc)

        with tc.tile_pool(name="scps", bufs=2, space="PSUM") as psc, \
             tc.tile_pool(name="cxps", bufs=1, space="PSUM") as pcx, \
             tc.tile_pool(name="pjps", bufs=1, space="PSUM") as ppj, \
             tc.tile_pool(name="qpT", bufs=3) as pqpt, \
             tc.tile_pool(name="attn", bufs=4) as patn, \
             tc.tile_pool(name="den", bufs=2) as pden, \
             tc.tile_pool(name="qpev", bufs=2) as pqpe, \
             tc.tile_pool(name="norm", bufs=2) as pnm, \
             tc.tile_pool(name="ctmp", bufs=2) as ptmp:
            qpT_tiles = {}

            def make_proj_thunks(jj):
                """Q^T proj + residual proj for pair jj as single-matmul
                thunks, consumed one per attention kb-step so TensorE slack
                is filled without stalling the exp stream."""
                state = {}
                thunks = []

                def qps_mm(d, n0, nn):
                    def f():
                        if 'qps' not in state:
                            state['qps'] = ppj.tile([128, T], F32, tag="pj",
                                                    name="pjq")
                        nc.tensor.matmul(
                            state['qps'][:, n0:n0 + nn],
                            lhsT=wq_sb[d][:, jj * 128:(jj + 1) * 128],
                            rhs=qx_sb[d][:, n0:n0 + nn],
                            start=(d == 0), stop=(d == DB - 1))
                    return f

                def qpt_evac():
                    qt = pqpt.tile([128, T], BF16, tag="qpT_t", name="qpT_t")
                    nc.vector.tensor_scalar(out=qt, in0=state['qps'],
                                            scalar1=bqT[:, jj:jj + 1],
                                            scalar2=None, op0=ALU.add)
                    qpT_tiles[jj] = qt

                def nps_mm(d, n0, nn):
                    def f():
                        if 'nps' not in state:
                            state['nps'] = ppj.tile([128, D], F32, tag="pj",
                                                    name="pjn")
                        nc.tensor.matmul(
                            state['nps'][:, n0:n0 + nn],
                            lhsT=qx_sb[d][:, jj * 128:(jj + 1) * 128],
                            rhs=wq_sb[d][:, n0:n0 + nn],
                            start=(d == 0), stop=(d == DB - 1))
                    return f

                def qp_evac():
                    ev = pqpe.tile([128, D], F32, tag="qp_ev", name="qp_ev")
                    nc.vector.tensor_add(out=ev, in0=state['nps'], in1=bqfc_bc)
                    nc.sync.dma_start(out=qp_dram[jj * 128:(jj + 1) * 128, :],
                                      in_=ev)

                for d in range(DB):
                    for n0, nn in nsplits(T):
                        thunks.append(qps_mm(d, n0, nn))
                thunks.append(qpt_evac)
                for d in range(DB):
                    for n0, nn in nsplits(D):
                        thunks.append(nps_mm(d, n0, nn))
                thunks.append(qp_evac)
                return thunks

            # pair 0's projections run during the V phase / attention ramp
            for th in make_proj_thunks(0):
                th()

            for j in range(PAIRS):
                qpT_t = qpT_tiles.pop(j)
                pending = make_proj_thunks(j + 1) if j + 1 < PAIRS else []
                for hh in range(2):
                    h = 2 * j + hh
                    pr = slice(hh * 64, hh * 64 + 64)
                    cx = pcx.tile([VW, T], F32, tag="cx", name="cx")
                    for kb in range(SB):
                        sc = psc.tile([128, T], F32, tag="sc", name="sc")
                        for n0, nn in nsplits(T):
                            nc.tensor.matmul(
                                sc[:, n0:n0 + nn],
                                lhsT=kpT_sb[j][pr, kb * 128:(kb + 1) * 128],
                                rhs=qpT_t[pr, n0:n0 + nn],
                                start=True, stop=True)
                        at = patn.tile([128, T], BF16, tag="at", name="at")
                        nc.scalar.activation(out=at, in_=sc, func=AF.Exp,
                                             scale=0.125)
                        vr = vp_sb[kb].rearrange("p (h c) -> p h c", c=VW)
                        for n0, nn in nsplits(T):
                            nc.tensor.matmul(
                                cx[:, n0:n0 + nn],
                                lhsT=vr[:, h, :],
                                rhs=at[:, n0:n0 + nn],
                                start=(kb == 0), stop=(kb == SB - 1))
                        if pending:
                            pending.pop(0)()
                    # evacuate unnormalized ctx + denominator
                    den = pden.tile([VW, T], F32, tag="den", name="den")
                    nc.vector.tensor_copy(out=den[64:65, :], in_=cx[64:65, :])
                    nc.gpsimd.dma_start(out=den_dram[h, :], in_=den[64:65, :])
                    if hh == 0:
                        nc.vector.tensor_copy(out=ctxT_sb[j][0:64, :],
                                              in_=cx[0:64, :])
                    else:
                        tmp = ptmp.tile([64, T], BF16, tag="ctmp", name="ctmp")
                        nc.vector.tensor_copy(out=tmp, in_=cx[0:64, :])
                        nc.sync.dma_start(out=ctxT_sb[j][64:128, :], in_=tmp)
                while pending:
                    pending.pop(0)()
                # normalize this pair's ctxT (cheap chain, off critical path)
                dbc = pnm.tile([128, T], F32, tag="dbc", name="dbc")
                nc.gpsimd.dma_start(
                    out=dbc[0:64, :],
                    in_=bcast_ap(den_dram[2 * j:2 * j + 1, :], 64))
                nc.gpsimd.dma_start(
                    out=dbc[64:128, :],
                    in_=bcast_ap(den_dram[2 * j + 1:2 * j + 2, :], 64))
                rbc = pnm.tile([128, T], F32, tag="rbc", name="rbc")
                nc.vector.reciprocal_approx_fast(out=rbc, in_=dbc)
                nc.vector.tensor_mul(out=ctxT_sb[j], in0=ctxT_sb[j], in1=rbc)

        # ================= FC + residual + layernorm ====================
        with tc.tile_pool(name="fcps", bufs=2, space="PSUM") as pfc, \
             tc.tile_pool(name="lnbc", bufs=1) as plnb, \
             tc.tile_pool(name="qpl", bufs=2) as pqp, \
             tc.tile_pool(name="xln", bufs=2) as px, \
             tc.tile_pool(name="stat", bufs=4) as pst:
            gamma_bc = plnb.tile([128, D], F32, tag="gamma_bc", name="gamma_bc")
            nc.gpsimd.dma_start(out=gamma_bc, in_=bcast_ap(gamma, 128))
            beta_bc = plnb.tile([128, D], F32, tag="beta_bc", name="beta_bc")
            nc.gpsimd.dma_start(out=beta_bc, in_=bcast_ap(beta, 128))

            for t in range(TB):
                qp_t = pqp.tile([128, D], F32, tag="qp_t", name="qp_t")
                nc.sync.dma_start(out=qp_t,
                                  in_=qp_dram[t * 128:(t + 1) * 128, :])
                fc = pfc.tile([128, D], F32, tag="fc", name="fc")
                for j in range(EB):
                    for n0, nn in nsplits(D):
                        nc.tensor.matmul(
                            fc[:, n0:n0 + nn],
                            lhsT=ctxT_sb[j][:, t * 128:(t + 1) * 128],
                            rhs=wfc_sb[j][:, n0:n0 + nn],
                            start=(j == 0), stop=(j == EB - 1))
                x = px.tile([128, D], F32, tag="x", name="x")
                nc.vector.tensor_add(out=x, in0=fc, in1=qp_t)
                ngr = max(D // 512, 1)
                gsz = min(D, 512)
                stats = pst.tile([128, ngr, 6], F32, tag="stats", name="stats")
                for g in range(ngr):
                    nc.vector.bn_stats(out=stats[:, g, :],
                                       in_=x[:, g * gsz:(g + 1) * gsz])
                mv = pst.tile([128, 2], F32, tag="mv", name="mv")
                nc.vector.bn_aggr(out=mv, in_=stats)
                rstd = pst.tile([128, 1], F32, tag="rstd", name="rstd")
                nc.scalar.activation(out=rstd, in_=mv[:, 1:2], func=AF.Sqrt,
                                     bias=eps_t, scale=1.0)
                nc.vector.reciprocal(out=rstd, in_=rstd)
                xn = px.tile([128, D], F32, tag="xn", name="xn")
                nc.vector.tensor_scalar(out=xn, in0=x, scalar1=mv[:, 0:1],
                                        scalar2=rstd, op0=ALU.subtract,
                                        op1=ALU.mult)
                nc.vector.tensor_mul(out=xn, in0=xn, in1=gamma_bc)
                nc.gpsimd.tensor_add(out=xn, in0=xn, in1=beta_bc)
                nc.sync.dma_start(out=out[t * 128:(t + 1) * 128, :], in_=xn)

    nc.compile()
    return nc


_B, _S, _D, _H, _DK = 4, 2048, 1024, 16, 64
_T = _S // 2
_NCORES = 8
_BF = ml_dtypes.bfloat16

_nc_cache = [None]


def _get_nc():
    if _nc_cache[0] is None:
        _nc_cache[0] = build(T=_T, S=_S, D=_D, H=_H, DK=_DK, n_cores=_NCORES)
    return _nc_cache[0]


def _execute(inputs, trace=False, tmpdir=None):
    from concourse.bass_utils import run_bass_kernel_spmd

    nc = _get_nc()
    q = np.asarray(inputs["q"], np.float32)
    k = np.asarray(inputs["k"], np.float32)
    v = np.asarray(inputs["v"], np.float32)
    Wq = np.asarray(inputs["Wq"], np.float32).astype(_BF)
    Wk = np.asarray(inputs["Wk"], np.float32).astype(_BF)
    Wv = np.asarray(inputs["Wv"], np.float32).astype(_BF)
    Wfc = np.asarray(inputs["Wfc"], np.float32).astype(_BF)
    fp = {n: np.asarray(inputs[n], np.float32)
          for n in ("bq", "bk", "bv", "bfc", "gamma", "beta")}

    in_maps = []
    for c in range(_NCORES):
        b, half = divmod(c, 2)
        t0 = half * _T
        in_maps.append({
            "qT": np.ascontiguousarray(q[b, t0:t0 + _T].T).astype(_BF),
            "kT": np.ascontiguousarray(k[b].T).astype(_BF),
            "vT": np.ascontiguousarray(v[b].T).astype(_BF),
            "Wq": Wq, "Wk": Wk, "Wv": Wv, "Wfc": Wfc, **fp,
        })

    res = run_bass_kernel_spmd(nc, in_maps, core_ids=list(range(_NCORES)),
                               trace=trace, tmpdir=tmpdir)
    out = np.empty((_B, _S, _D), np.float32)
    for c in range(_NCORES):
        b, half = divmod(c, 2)
        out[b, half * _T:(half + 1) * _T] = res.results[c]["out"]
    return out, res.exec_time_ns


def kernel(**inputs) -> np.ndarray:
    out, _ = _execute(inputs, trace=False)
    return out



# revision 19
# speedup vs baseline: 1.0256x; 1.0256x over previous
"""Trainium2 Bass kernel for nn_AttentionLayer (B=4, S=2048, D=1024, H=16).

Self-contained: builds and compiles an SPMD Bass/Tile program once, then
runs it across 8 NeuronCores via run_bass_kernel_spmd.

Sharding (no collectives): core c handles batch b = c // 2 and query-token
half c % 2 (1024 query tokens). Each core receives pre-transposed bf16
activations (x^T slices) plus bf16 weights, computes its [1024, 1024]
slice of the final layernorm output in fp32, and the host reassembles.

Per-core pipeline (all matmuls bf16 with fp32 PSUM accumulation):
- K projection with chunked double-buffered kT loads, then V projection
  into a kb-paired vp layout [128, 2, H, 65] (64 v dims + ones column per
  head so attn@V also produces the softmax denominator row).
- Attention per head-pair: scores^T = Kh @ Qh^T into double-buffered
  PSUM, exp on ScalarE into paired at tiles [128, 2, T], attn@V
  accumulation; the next pair's Q^T projection is emitted one matmul per
  step to fill TensorE slack inside the ACT-bound loop.
- Softmax denominators stay on-chip: GPSIMD partition-broadcast of the
  ones-column row, fast reciprocal, one multiply per pair.
- FC folds the residual: x = ctx@Wfc + q@Wq + (bq+bfc) as one K=2048+1
  PSUM accumulation per 128-token block, layernorm reads PSUM directly.
"""

import numpy as np
import ml_dtypes


from contextlib import ExitStack

import concourse.bass as bass
import concourse.tile as tile
import concourse.mybir as mybir
from concourse import bacc

F32 = mybir.dt.float32
BF16 = mybir.dt.bfloat16
AF = mybir.ActivationFunctionType
ALU = mybir.AluOpType


def bcast_ap(ap: bass.AP, parts: int) -> bass.AP:
    """Partition-broadcast a [1, N]-shaped DRAM AP to [parts, N]."""
    return bass.AP(tensor=ap.tensor, offset=ap.offset,
                   ap=[[0, parts]] + list(ap.ap[-1:]))


def nsplits(total, cap=512):
    return [(i, min(cap, total - i)) for i in range(0, total, cap)]


def build(T=1024, S=2048, D=1024, H=16, DK=64, n_cores=8, eps=1e-5,
          trn_type="TRN2", debug=False):
    assert DK == 64 and H % 2 == 0 and D == H * DK
    DB = D // 128     # contraction chunks over d
    EB = D // 128     # e blocks (projection output chunks); == H//2
    TB = T // 128
    SB = S // 128
    S2 = SB // 2      # 256-token kv blocks
    PAIRS = H // 2
    VW = 65           # per-head vp stripe: 64 v columns + 1 ones column

    nc = bacc.Bacc(trn_type, target_bir_lowering=False, debug=False,
                   num_devices=n_cores)

    qT = nc.dram_tensor("qT", [D, T], BF16, kind="ExternalInput").ap()
    kT = nc.dram_tensor("kT", [D, S], BF16, kind="ExternalInput").ap()
    vT = nc.dram_tensor("vT", [D, S], BF16, kind="ExternalInput").ap()
    Wq = nc.dram_tensor("Wq", [D, D], BF16, kind="ExternalInput").ap()
    Wk = nc.dram_tensor("Wk", [D, D], BF16, kind="ExternalInput").ap()
    Wv = nc.dram_tensor("Wv", [D, D], BF16, kind="ExternalInput").ap()
    Wfc = nc.dram_tensor("Wfc", [D, D], BF16, kind="ExternalInput").ap()
    bq = nc.dram_tensor("bq", [D], F32, kind="ExternalInput").ap()
    bk = nc.dram_tensor("bk", [D], F32, kind="ExternalInput").ap()
    bv = nc.dram_tensor("bv", [D], F32, kind="ExternalInput").ap()
    bfc = nc.dram_tensor("bfc", [D], F32, kind="ExternalInput").ap()
    gamma = nc.dram_tensor("gamma", [D], F32, kind="ExternalInput").ap()
    beta = nc.dram_tensor("beta", [D], F32, kind="ExternalInput").ap()
    out = nc.dram_tensor("out", [T, D], F32, kind="ExternalOutput").ap()
    den_dram = nc.dram_tensor("den_scratch", [H, T], F32).ap()
    dbg = {}
    if debug:
        for nm, shape in [("dbg_qx", [128, T]), ("dbg_wq", [128, D]),
                          ("dbg_qpT", [128, T]), ("dbg_kpT", [128, S]),
                          ("dbg_vp", [128, 2 * H * VW]),
                          ("dbg_ctxT", [128, T]), ("dbg_at", [128, 2 * T])]:
            dbg[nm] = nc.dram_tensor(nm, shape, BF16,
                                     kind="ExternalOutput").ap()

    with tile.TileContext(nc) as tc, ExitStack() as ctx:
        pconst = ctx.enter_context(tc.tile_pool(name="const", bufs=1))
        ppers = ctx.enter_context(tc.tile_pool(name="persist", bufs=1))

        # ---- tiny constants (gpsimd DMA queue) ------------------------
        bqT = pconst.tile([128, EB], F32, tag="bqT", name="bqT")
        nc.gpsimd.dma_start(out=bqT, in_=bq.rearrange("(e p) -> p e", p=128))
        bkT = pconst.tile([128, EB], F32, tag="bkT", name="bkT")
        nc.gpsimd.dma_start(out=bkT, in_=bk.rearrange("(e p) -> p e", p=128))
        eps_t = pconst.tile([128, 1], F32, tag="eps", name="eps")
        nc.vector.memset(eps_t, eps)
        ones_1 = pconst.tile([1, 128], BF16, tag="ones1", name="ones1")
        nc.vector.memset(ones_1, 1.0)
        # (bq + bfc) as a [1, D] bf16 row for the FC K=1 bias matmul
        bq_row = pconst.tile([1, D], F32, tag="bq_row", name="bq_row")
        nc.gpsimd.dma_start(out=bq_row, in_=bq.rearrange("(o n) -> o n", o=1))
        bfc_row = pconst.tile([1, D], F32, tag="bfc_row", name="bfc_row")
        nc.gpsimd.dma_start(out=bfc_row, in_=bfc.rearrange("(o n) -> o n", o=1))
        bqfc_n = pconst.tile([1, D], BF16, tag="bqfc_n", name="bqfc_n")
        nc.vector.tensor_add(out=bqfc_n, in0=bq_row, in1=bfc_row)
        bv_bc = pconst.tile([128, D], F32, tag="bv_bc", name="bv_bc")
        nc.gpsimd.dma_start(out=bv_bc, in_=bcast_ap(bv, 128))
        gamma_bc = pconst.tile([128, D], F32, tag="gamma_bc", name="gamma_bc")
        nc.gpsimd.dma_start(out=gamma_bc, in_=bcast_ap(gamma, 128))
        beta_bc = pconst.tile([128, D], F32, tag="beta_bc", name="beta_bc")
        nc.gpsimd.dma_start(out=beta_bc, in_=bcast_ap(beta, 128))

        # ---- persistent SBUF ------------------------------------------
        kpT_sb = [ppers.tile([128, S], BF16, tag=f"kpT{e}", name=f"kpT{e}")
                  for e in range(EB)]
        vp2_sb = [ppers.tile([128, 2 * H * VW], BF16, tag=f"vp{s2}",
                             name=f"vp{s2}") for s2 in range(S2)]
        ctxT_sb = [ppers.tile([128, T], BF16, tag=f"ctxT{e}", name=f"ctxT{e}")
                   for e in range(EB)]

        pqx = ctx.enter_context(tc.tile_pool(name="qx", bufs=1))
        pwq = ctx.enter_context(tc.tile_pool(name="wq", bufs=1))
        qx_sb = [pqx.tile([128, T], BF16, tag=f"qx{d}", name=f"qx{d}")
                 for d in range(DB)]
        wq_sb = [pwq.tile([128, D], BF16, tag=f"wq{d}", name=f"wq{d}")
                 for d in range(DB)]

        # ============ K projection + V projection ======================
        CK = 512
        NCK = len(nsplits(S, CK))
        with tc.tile_pool(name="wk", bufs=1) as pwk, \
             tc.tile_pool(name="kx", bufs=2 * DB) as pkx, \
             tc.tile_pool(name="wv", bufs=1) as pwv, \
             tc.tile_pool(name="vx", bufs=2 * DB) as pvx, \
             tc.tile_pool(name="kps", bufs=2, space="PSUM") as pkps, \
             tc.tile_pool(name="vps", bufs=2, space="PSUM") as pvps:
            # first chunk: interleave wk[d] with kx[d] so the d=0 matmul can
            # start after just two transfers; later chunks follow.
            wk_sb = [pwk.tile([128, D], BF16, tag=f"wk{d}", name=f"wk{d}")
                     for d in range(DB)]
            kx_t = [[pkx.tile([128, CK], BF16, tag="kx", name=f"kx{d}_{ci}")
                     for d in range(DB)] for ci in range(NCK)]
            c0s = nsplits(S, CK)
            for d in range(DB):
                nc.sync.dma_start(out=wk_sb[d], in_=Wk[d * 128:(d + 1) * 128, :])
                c0, cn = c0s[0]
                nc.sync.dma_start(out=kx_t[0][d][:, 0:cn],
                                  in_=kT[d * 128:(d + 1) * 128, c0:c0 + cn])
            for ci, (c0, cn) in list(enumerate(c0s))[1:]:
                for d in range(DB):
                    nc.sync.dma_start(out=kx_t[ci][d][:, 0:cn],
                                      in_=kT[d * 128:(d + 1) * 128, c0:c0 + cn])
            # loads needed later, in consumption order
            for d in range(DB):
                nc.sync.dma_start(out=qx_sb[d], in_=qT[d * 128:(d + 1) * 128, :])
                nc.sync.dma_start(out=wq_sb[d], in_=Wq[d * 128:(d + 1) * 128, :])
            wv_sb = [pwv.tile([128, D], BF16, tag=f"wv{d}", name=f"wv{d}")
                     for d in range(DB)]
            for d in range(DB):
                nc.sync.dma_start(out=wv_sb[d],
                                  in_=Wv[d * 128:(d + 1) * 128, :])
            vx_t = []
            for ci, (c0, cn) in enumerate(nsplits(S, CK)):
                tiles = [pvx.tile([128, CK], BF16, tag="vx",
                                  name=f"vx{d}_{ci}") for d in range(DB)]
                for d in range(DB):
                    nc.sync.dma_start(out=tiles[d][:, 0:cn],
                                      in_=vT[d * 128:(d + 1) * 128, c0:c0 + cn])
                vx_t.append(tiles)

            # K projection: kpT[e][:, chunk] = (Wk^T k^T + bk) in bf16
            for ci, (c0, cn) in enumerate(nsplits(S, CK)):
                for e in range(EB):
                    ps = pkps.tile([128, CK], F32, tag="kps", name="kps")
                    for d in range(DB):
                        nc.tensor.matmul(
                            ps[:, 0:cn],
                            lhsT=wk_sb[d][:, e * 128:(e + 1) * 128],
                            rhs=kx_t[ci][d][:, 0:cn],
                            start=(d == 0), stop=(d == DB - 1))
                    nc.vector.tensor_scalar(
                        out=kpT_sb[e][:, c0:c0 + cn], in0=ps[:, 0:cn],
                        scalar1=bkT[:, e:e + 1], scalar2=None, op0=ALU.add)

            # V projection into paired layout vp2[s2][:, i, h, 0:64] + ones
            SPC = CK // 128   # 128-token blocks per vx chunk
            for s in range(SB):
                ps = pvps.tile([128, D], F32, tag="vps", name="vps")
                for n0, nn in nsplits(D):
                    for d in range(DB):
                        nc.tensor.matmul(
                            ps[:, n0:n0 + nn],
                            lhsT=vx_t[s // SPC][d][:, (s % SPC) * 128:
                                                   (s % SPC + 1) * 128],
                            rhs=wv_sb[d][:, n0:n0 + nn],
                            start=(d == 0), stop=(d == DB - 1))
                vr = vp2_sb[s // 2].rearrange("p (i h c) -> p i h c",
                                              i=2, c=VW)
                nc.vector.tensor_add(
                    out=vr[:, s % 2, :, 0:DK],
                    in0=ps.rearrange("p (h c) -> p h c", c=DK),
                    in1=bv_bc.rearrange("p (h c) -> p h c", c=DK))
                nc.vector.memset(vr[:, s % 2, :, DK:VW], 1.0)

        # wfc lives in space freed by the K/V-phase pools; its DMA runs
        # during the attention phase, well before the FC consumer.
        pwfc = ctx.enter_context(tc.tile_pool(name="wfc", bufs=1))
        wfc_sb = [pwfc.tile([128, D], BF16, tag=f"wfc{d}", name=f"wfc{d}")
                  for d in range(DB)]
        for d in range(DB):
            nc.gpsimd.dma_start(out=wfc_sb[d], in_=Wfc[d * 128:(d + 1) * 128, :])

        # ================= attention (+ Q-proj interleaved) =============
        with tc.tile_pool(name="scps", bufs=2, space="PSUM") as psc, \
             tc.tile_pool(name="cxps", bufs=1, space="PSUM") as pcx, \
             tc.tile_pool(name="pjps", bufs=1, space="PSUM") as ppj, \
             tc.tile_pool(name="qpT", bufs=2) as pqpt, \
             tc.tile_pool(name="attn", bufs=3) as patn, \
             tc.tile_pool(name="norm", bufs=2) as pnm, \
             tc.tile_pool(name="ctmp", bufs=2) as ptmp:
            qpT_tiles = {}

            def make_proj_thunks(jj):
                """Q^T projection for pair jj as single-matmul thunks,
                consumed one per attention step so TensorE slack is filled
                without stalling the exp stream."""
                state = {}
                thunks = []

                def qps_mm(d, n0, nn):
                    def f():
                        if 'qps' not in state:
                            state['qps'] = ppj.tile([128, T], F32, tag="pj",
                                                    name="pjq")
                        nc.tensor.matmul(
                            state['qps'][:, n0:n0 + nn],
                            lhsT=wq_sb[d][:, jj * 128:(jj + 1) * 128],
                            rhs=qx_sb[d][:, n0:n0 + nn],
                            start=(d == 0), stop=(d == DB - 1))
                    return f

                def qpt_evac():
                    qt = pqpt.tile([128, T], BF16, tag="qpT_t", name="qpT_t")
                    nc.vector.tensor_scalar(out=qt, in0=state['qps'],
                                            scalar1=bqT[:, jj:jj + 1],
                                            scalar2=None, op0=ALU.add)
                    qpT_tiles[jj] = qt

                for d in range(DB):
                    for n0, nn in nsplits(T):
                        thunks.append(qps_mm(d, n0, nn))
                thunks.append(qpt_evac)
                return thunks

            # pair 0's projection runs during the V phase ramp
            for th in make_proj_thunks(0):
                th()

            for j in range(PAIRS):
                qpT_t = qpT_tiles.pop(j)
                if debug and j == 0:
                    nc.sync.dma_start(out=dbg["dbg_qpT"], in_=qpT_t)
                pending = make_proj_thunks(j + 1) if j + 1 < PAIRS else []
                dbc = pnm.tile([128, T], F32, tag="dbc", name="dbc")
                for hh in range(2):
                    h = 2 * j + hh
                    pr = slice(hh * 64, hh * 64 + 64)
                    cx = pcx.tile([VW, T], F32, tag="cx", name="cx")
                    for kbp in range(S2):
                        at2 = patn.tile([128, 2 * T], BF16, tag="at",
                                        name="at")
                        a2 = at2.rearrange("p (i n) -> p i n", i=2)
                        for i in range(2):
                            kb = 2 * kbp + i
                            sc = psc.tile([128, T], F32, tag="sc", name="sc")
                            for n0, nn in nsplits(T):
                                nc.tensor.matmul(
                                    sc[:, n0:n0 + nn],
                                    lhsT=kpT_sb[j][pr, kb * 128:(kb + 1) * 128],
                                    rhs=qpT_t[pr, n0:n0 + nn],
                                    start=True, stop=True)
                            nc.scalar.activation(out=a2[:, i, :], in_=sc,
                                                 func=AF.Exp, scale=0.125)
                            if pending:
                                pending.pop(0)()
                        vr = vp2_sb[kbp].rearrange("p (i h c) -> p i h c",
                                                   i=2, c=VW)
                        for i in range(2):
                            for n0, nn in nsplits(T):
                                nc.tensor.matmul(
                                    cx[:, n0:n0 + nn],
                                    lhsT=vr[:, i, h, :],
                                    rhs=a2[:, i, n0:n0 + nn],
                                    start=(kbp == 0 and i == 0),
                                    stop=(kbp == S2 - 1 and i == 1))
                        if debug and j == 0 and hh == 0 and kbp == 0:
                            nc.sync.dma_start(out=dbg["dbg_at"], in_=at2)
                    # evacuate unnormalized ctx + denominator; the den row
                    # crosses partitions, so it goes via a DRAM roundtrip
                    # (broadcast-DMA back) like the baseline.
                    den = pnm.tile([VW, T], F32, tag="den", name="den")
                    nc.vector.tensor_copy(out=den[DK:VW, :], in_=cx[DK:VW, :])
                    nc.gpsimd.dma_start(out=den_dram[h, :], in_=den[DK:VW, :])
                    nc.gpsimd.dma_start(
                        out=dbc[hh * 64:(hh + 1) * 64, :],
                        in_=bcast_ap(den_dram[h:h + 1, :], 64))
                    if hh == 0:
                        nc.vector.tensor_copy(out=ctxT_sb[j][0:64, :],
                                              in_=cx[0:DK, :])
                    else:
                        tmp = ptmp.tile([64, T], BF16, tag="ctmp", name="ctmp")
                        nc.vector.tensor_copy(out=tmp, in_=cx[0:DK, :])
                        nc.sync.dma_start(out=ctxT_sb[j][64:128, :], in_=tmp)
                while pending:
                    pending.pop(0)()
                # normalize this pair's ctxT (off the ACT critical path)
                rbc = pnm.tile([128, T], F32, tag="rbc", name="rbc")
                nc.vector.reciprocal_approx_fast(out=rbc, in_=dbc)
                nc.vector.tensor_mul(out=ctxT_sb[j], in0=ctxT_sb[j], in1=rbc)

        # ========= FC (+ folded residual + bias) + layernorm ===========
        with tc.tile_pool(name="fcps", bufs=2, space="PSUM") as pfc, \
             tc.tile_pool(name="xln", bufs=2) as px, \
             tc.tile_pool(name="stat", bufs=4) as pst:
            for t in range(TB):
                fc = pfc.tile([128, D], F32, tag="fc", name="fc")
                for n0, nn in nsplits(D):
                    nc.tensor.matmul(
                        fc[:, n0:n0 + nn], lhsT=ones_1,
                        rhs=bqfc_n[0:1, n0:n0 + nn],
                        start=True, stop=False)
                    for j in range(EB):
                        nc.tensor.matmul(
                            fc[:, n0:n0 + nn],
                            lhsT=ctxT_sb[j][:, t * 128:(t + 1) * 128],
                            rhs=wfc_sb[j][:, n0:n0 + nn],
                            start=False, stop=False)
                    for d in range(DB):
                        nc.tensor.matmul(
                            fc[:, n0:n0 + nn],
                            lhsT=qx_sb[d][:, t * 128:(t + 1) * 128],
                            rhs=wq_sb[d][:, n0:n0 + nn],
                            start=False, stop=(d == DB - 1))
                ngr = max(D // 512, 1)
                gsz = min(D, 512)
                stats = pst.tile([128, ngr, 6], F32, tag="stats", name="stats")
                for g in range(ngr):
                    nc.vector.bn_stats(out=stats[:, g, :],
                                       in_=fc[:, g * gsz:(g + 1) * gsz])
                mv = pst.tile([128, 2], F32, tag="mv", name="mv")
                nc.vector.bn_aggr(out=mv, in_=stats)
                rstd = pst.tile([128, 1], F32, tag="rstd", name="rstd")
                nc.scalar.activation(out=rstd, in_=mv[:, 1:2], func=AF.Sqrt,
                                     bias=eps_t, scale=1.0)
                nc.vector.reciprocal(out=rstd, in_=rstd)
                xn = px.tile([128, D], F32, tag="xn", name="xn")
                nc.vector.tensor_scalar(out=xn, in0=fc, scalar1=mv[:, 0:1],
                                        scalar2=rstd, op0=ALU.subtract,
                                        op1=ALU.mult)
                nc.vector.tensor_mul(out=xn, in0=xn, in1=gamma_bc)
                nc.gpsimd.tensor_add(out=xn, in0=xn, in1=beta_bc)
                nc.sync.dma_start(out=out[t * 128:(t + 1) * 128, :], in_=xn)

            if debug:
                nc.sync.dma_start(out=dbg["dbg_qx"], in_=qx_sb[0])
                nc.sync.dma_start(out=dbg["dbg_wq"], in_=wq_sb[0])
                nc.sync.dma_start(out=dbg["dbg_kpT"], in_=kpT_sb[0])
                nc.sync.dma_start(out=dbg["dbg_vp"], in_=vp2_sb[0])
                nc.sync.dma_start(out=dbg["dbg_ctxT"], in_=ctxT_sb[0])

    nc.compile()
    return nc


_B, _S, _D, _H, _DK = 4, 2048, 1024, 16, 64
_T = _S // 2
_NCORES = 8
_BF = ml_dtypes.bfloat16

_nc_cache = [None]


def _get_nc():
    if _nc_cache[0] is None:
        _nc_cache[0] = build(T=_T, S=_S, D=_D, H=_H, DK=_DK, n_cores=_NCORES)
    return _nc_cache[0]


def _make_in_maps(inputs):
    q = np.asarray(inputs["q"], np.float32)
    k = np.asarray(inputs["k"], np.float32)
    v = np.asarray(inputs["v"], np.float32)
    Wq = np.asarray(inputs["Wq"], np.float32).astype(_BF)
    Wk = np.asarray(inputs["Wk"], np.float32).astype(_BF)
    Wv = np.asarray(inputs["Wv"], np.float32).astype(_BF)
    Wfc = np.asarray(inputs["Wfc"], np.float32).astype(_BF)
    fp = {n: np.asarray(inputs[n], np.float32)
          for n in ("bq", "bk", "bv", "bfc", "gamma", "beta")}

    in_maps = []
    for c in range(_NCORES):
        b, half = divmod(c, 2)
        t0 = half * _T
        in_maps.append({
            "qT": np.ascontiguousarray(q[b, t0:t0 + _T].T).astype(_BF),
            "kT": np.ascontiguousarray(k[b].T).astype(_BF),
            "vT": np.ascontiguousarray(v[b].T).astype(_BF),
            "Wq": Wq, "Wk": Wk, "Wv": Wv, "Wfc": Wfc, **fp,
        })
    return in_maps


def _execute(inputs, trace=False, tmpdir=None):
    from concourse.bass_utils import run_bass_kernel_spmd

    nc = _get_nc()
    in_maps = _make_in_maps(inputs)
    res = run_bass_kernel_spmd(nc, in_maps, core_ids=list(range(_NCORES)),
                               trace=trace, tmpdir=tmpdir)
    out = np.empty((_B, _S, _D), np.float32)
    for c in range(_NCORES):
        b, half = divmod(c, 2)
        out[b, half * _T:(half + 1) * _T] = res.results[c]["out"]
    return out, res.exec_time_ns


def kernel(**inputs) -> np.ndarray:
    out, _ = _execute(inputs, trace=False)
    return out
